# revision 50
# baseline (speedup 1.0000x reference)
"""AttentionBlock (GroupNorm + single-head full attention + residual) on 8
Trainium2 NeuronCores.

Sharding: data-parallel over batch (4) x sequence-parallel over query
tokens (2 halves of h*w=4096). Each core gets its batch slice with the
token axis ROTATED by the host so that its 2048 queries are always
columns 0:NQ (attention is permutation-invariant over keys, GroupNorm
over positions), so a single xb input serves stats, K, V and the query
slice. No collectives; the host scatters inputs and gathers outputs.

Per-core pipeline (channels on partitions, [256 = 2x128p, n] tiles; all
big matmuls in float32r = full PE rate at N>=256 with ~TF32 accuracy):
 - GroupNorm stats: per-channel sum (DVE reduce) and sum-of-squares
   (ACT Square with accum_out) per 1024-column chunk as DMAs land;
   cross-partition group reduction and broadcast back to channels via
   tiny indicator matmuls on the PE.
 - The normalize pass is FOLDED INTO THE WEIGHTS: W' = W * a per input
   channel (a = gamma*rstd) and bias' = W @ b + bias (b = beta-mean*a),
   so Q/K/V2 matmuls consume RAW x and no x_norm is ever materialized.
 - out_w is folded into V: W2 = out_w @ Wv on device, V2^T = x^T @ W2'^T
   directly -- no separate V stage or final projection matmul.
 - Attention per 512-query chunk: S^T[k,q] = K_tile^T.T @ Q_chunk;
   P^T = exp(S^T/16) on ScalarE (scores ~ N(0,1): no max-subtraction
   needed in fp32); out2[q,:] = sum_k P^T[:,q].T @ [V2^T | 1] where the
   appended ones column accumulates the softmax denominator l in the
   same matmuls. Rows are scaled by 1/l, PE-transposed back to [c, q],
   and added to the residual x + out_b + out_w @ bias_v + W2 @ b.

Toolchain notes: this walrus accepts at most one sync-wait per
instruction (SplitWaitTileContext splits the rest onto nops); float32r
requires producers to write f32r (gpsimd casting DMAs / engine copies)
and even innermost free counts (PV padded 257->258). fp8 DoubleRow for
PV was tried and is a net HW loss (per-matmul LDWEIGHTS churn).
"""

import numpy as np

B, C, HW = 4, 256, 4096
NQ = HW // 2
G = 8
CPG = C // G  # channels per group
EPS = 1e-5
N_CORES = 8
USE_FP8_PV = False
EXP_BIAS = -3.0

_CACHE = {}


def _build_nc(loop_reps=1, debug=False):
    import bass_rust
    import concourse.bass as bass
    import concourse.mybir as mybir
    import concourse.tile as tile
    from concourse.masks import make_identity
    from concourse.vector_clock import ScopedClock

    F32 = mybir.dt.float32
    FR = mybir.dt.float32r
    F8 = mybir.dt.float8e4
    AF = mybir.ActivationFunctionType
    ALU = mybir.AluOpType

    MAXW = 1

    class SplitWaitTileContext(tile.TileContext):
        """Workaround: this toolchain's walrus accepts at most one sync-wait
        per instruction; split excess waits onto same-engine InstNoOps."""

        def _split_excess_waits(self, inst):
            si = inst.sync_info
            if si is None:
                return []
            waits = list(si.on_wait)
            if len(waits) <= MAXW:
                return []
            extra, keep = waits[:-MAXW], waits[-MAXW:]
            nops = [
                mybir.InstNoOp(
                    name=f"I-{self.nc.next_id()}",
                    sync_info=mybir.SyncInfo(on_wait=[w], on_update=[]),
                    bass_nofuse=True,
                    engine=inst.engine,
                )
                for w in extra
            ]
            inst.sync_info = mybir.SyncInfo(on_wait=keep, on_update=list(si.on_update))
            return nops

        def _commit_and_lower(self, inst, original_block, old_bb_map, bb_to_exit_bb):
            for nop in self._split_excess_waits(inst):
                self._commit_instruction(nop, lazy_reg_writes=False)
            return super()._commit_and_lower(
                inst, original_block, old_bb_map, bb_to_exit_bb
            )

        def _drain_and_barrier(self, tick_clock, wait_clock):
            drain_inst = self.nc.sync.drain()
            wait_clock.add_sem_waits(
                drain_inst.ins, ScopedClock({None: tick_clock.global_clock})
            )
            si = drain_inst.ins.sync_info
            waits = list(si.on_wait) if si is not None else []
            if len(waits) > MAXW:
                updates = list(si.on_update) if si is not None else []
                drain_inst.ins.sync_info = bass_rust.SyncInfo(
                    on_wait=waits[:MAXW], on_update=[]
                )
                rest = waits[MAXW:]
                for i, w in enumerate(rest):
                    extra = self.nc.sync.drain()
                    extra.ins.sync_info = bass_rust.SyncInfo(
                        on_wait=[w], on_update=updates if i == len(rest) - 1 else []
                    )
            self.nc.all_engine_barrier()
            assert self.sems is not None
            popped = self.nc._tile_sem_poison_stack.pop()
            assert popped is self._sem_poison
            self.nc.clear_and_free_semaphores(list(self.sems.allocated().values()))
            self.nc.all_engine_barrier()

    nc = bass.Bass()
    xb = nc.dram_tensor("xb", [C, HW], F32, kind="ExternalInput")
    qkv_w = nc.dram_tensor("qkv_w", [3 * C, C], F32, kind="ExternalInput")
    qkv_b = nc.dram_tensor("qkv_b", [3 * C], F32, kind="ExternalInput")
    out_w = nc.dram_tensor("out_w", [C, C], F32, kind="ExternalInput")
    out_b = nc.dram_tensor("out_b", [C], F32, kind="ExternalInput")
    gn_gamma = nc.dram_tensor("gn_gamma", [C], F32, kind="ExternalInput")
    gn_beta = nc.dram_tensor("gn_beta", [C], F32, kind="ExternalInput")
    gind_in = nc.dram_tensor("gind_in", [128, 16], F32, kind="ExternalInput")
    hind_in = nc.dram_tensor("hind_in", [8, 128 * 2], F32, kind="ExternalInput")
    ones_in = nc.dram_tensor("ones_in", [128, 256], F32, kind="ExternalInput")
    ident_in = nc.dram_tensor("ident_in", [128, 128], F32, kind="ExternalInput")
    y = nc.dram_tensor("y", [C, NQ], F32, kind="ExternalOutput")
    if debug:
        d_xn = nc.dram_tensor("d_xn", [C, HW], F32, kind="ExternalOutput")
        d_q = nc.dram_tensor("d_q", [C, NQ], F32, kind="ExternalOutput")
        d_k = nc.dram_tensor("d_k", [C, HW], F32, kind="ExternalOutput")
        d_v2t = nc.dram_tensor("d_v2t", [HW, 272], F32, kind="ExternalOutput")
        d_po = nc.dram_tensor("d_po", [128, 272], F32, kind="ExternalOutput")
        d_ab = nc.dram_tensor("d_ab", [C, 2], F32, kind="ExternalOutput")

    with SplitWaitTileContext(nc) as tc:
        import contextlib

        ctx = contextlib.ExitStack()
        with ctx:
            singles = ctx.enter_context(tc.tile_pool(name="singles", bufs=1))
            xpool = ctx.enter_context(tc.tile_pool(name="xpool", bufs=2))
            qpool = ctx.enter_context(tc.tile_pool(name="qpool", bufs=2))
            kpool = ctx.enter_context(tc.tile_pool(name="kpool", bufs=2))
            vpool = ctx.enter_context(tc.tile_pool(name="vpool", bufs=2))
            ypool = ctx.enter_context(tc.tile_pool(name="ypool", bufs=2))
            wpool = ctx.enter_context(tc.tile_pool(name="wpool", bufs=1))
            wnat = ctx.enter_context(tc.tile_pool(name="wnat", bufs=1))
            ppool = ctx.enter_context(tc.tile_pool(name="ppool", bufs=3))
            opool = ctx.enter_context(tc.tile_pool(name="opool", bufs=3))
            small = ctx.enter_context(tc.tile_pool(name="small", bufs=4))
            stat = ctx.enter_context(tc.tile_pool(name="stat", bufs=2))
            scpool = ctx.enter_context(tc.tile_pool(name="scpool", bufs=2))
            psmm = ctx.enter_context(tc.tile_pool(name="psmm", bufs=4, space="PSUM"))
            psov = ctx.enter_context(tc.tile_pool(name="psov", bufs=4, space="PSUM"))

            def setup():
                # constants: allocate tiles; DMAs deferred so weight/x loads
                # hit the serial DMA-issue paths first.
                ident = singles.tile([128, 128], F32, tag="ident")
                nc.sync.dma_start(out=ident, in_=ident_in[:, :])
                eps_sb = singles.tile([8, 1], F32, tag="eps")
                nc.vector.memset(eps_sb, EPS)
                ebias_sb = singles.tile([128, 1], F32, tag="ebias")
                nc.vector.memset(ebias_sb, EXP_BIAS)
                gam_sb = singles.tile([128, 2], F32, tag="gam")
                bet_sb = singles.tile([128, 2], F32, tag="bet")
                qb_sb = singles.tile([128, 6], F32, tag="qb")
                ob_sb = singles.tile([128, 2], F32, tag="ob")
                gi_sb = singles.tile([128, 16], F32, tag="gi")
                hi_sb = singles.tile([8, 128 * 2], F32, tag="hi")

                def load_consts():
                    nc.sync.dma_start(out=qb_sb, in_=qkv_b.rearrange("(m p) -> p m", p=128))
                    nc.sync.dma_start(out=gam_sb, in_=gn_gamma.rearrange("(t p) -> p t", p=128))
                    nc.sync.dma_start(out=bet_sb, in_=gn_beta.rearrange("(t p) -> p t", p=128))
                    nc.sync.dma_start(out=ob_sb, in_=out_b.rearrange("(t p) -> p t", p=128))
                    nc.sync.dma_start(out=gi_sb, in_=gind_in[:, :])
                    nc.sync.dma_start(out=hi_sb, in_=hind_in[:, :])

                g_sb = [gam_sb[:, t : t + 1] for t in range(2)]
                be_sb = [bet_sb[:, t : t + 1] for t in range(2)]
                gind = [gi_sb[:, 0:8], gi_sb[:, 8:16]]
                hind = [hi_sb[:, 0:128], hi_sb[:, 128:256]]
                return (ident, g_sb, be_sb, qb_sb, ob_sb, eps_sb, ebias_sb, gind, hind, load_consts)

            def body(rep, consts):
                (ident, g_sb, be_sb, qb_sb, ob_sb, eps_sb, ebias_sb, gind, hind, load_consts) = consts
                # ---------- transpose weights ----------
                wT = []  # (Wq|Wk)^T tiles [c_in 128, 512] f32r
                for t in range(2):
                    wT.append(wpool.tile([128, 512], FR, tag=f"wT{t}", name=f"wTn{t}"))
                owT = []  # out_w^T tiles [c_in 128, 256] f32r
                for t in range(2):
                    owT.append(wpool.tile([128, 256], FR, tag=f"owT{t}", name=f"owT{t}"))
                wns = []
                for i in range(6):
                    src = qkv_w if i < 4 else out_w
                    r0 = i * 128 if i < 4 else (i - 4) * 128
                    wn = wnat.tile([128, C], F32, tag=f"wn{i}", name=f"wn{i}")
                    nc.sync.dma_start(out=wn, in_=src[r0 : r0 + 128, :])
                    wns.append(wn)

                # x (gpsimd, 2 chunks/tile) + xq (sync, 1/tile) + const DMAs
                x_sb = []
                for t in range(2):
                    xt = xpool.tile([128, HW], FR, tag="xv", name=f"x{t}")
                    for c4 in range(4):
                        nc.gpsimd.dma_start(
                            out=xt[:, c4 * 1024 : (c4 + 1) * 1024],
                            in_=xb[t * 128 : (t + 1) * 128, c4 * 1024 : (c4 + 1) * 1024],
                        )
                    x_sb.append(xt)
                # host pre-rotates xb per core so the query half is always
                # columns 0:NQ (attention is permutation-invariant over keys)
                xq_sb = [x_sb[t][:, 0:NQ] for t in range(2)]
                load_consts()
                for i in range(6):
                    wn = wns[i]
                    dstT = wT if i < 4 else owT
                    col = i if i < 4 else i - 4
                    for t in range(2):
                        pst = psmm.tile([128, 128], F32, tag="mm")
                        nc.tensor.transpose(pst, wn[:, t * 128 : (t + 1) * 128], ident)
                        if (i + t) % 2 == 0:
                            nc.vector.tensor_copy(dstT[t][:, col * 128 : (col + 1) * 128], pst)
                        else:
                            nc.scalar.copy(dstT[t][:, col * 128 : (col + 1) * 128], pst)

                # ---------- W2 = out_w @ Wv fold ----------
                wv_fr = []
                for i in range(2):
                    wv = wpool.tile([128, C], FR, tag=f"wv{i}", name=f"wv{i}")
                    nc.gpsimd.dma_start(
                        out=wv, in_=qkv_w[512 + i * 128 : 512 + (i + 1) * 128, :]
                    )
                    wv_fr.append(wv)
                w2t = []
                for t in range(2):
                    ps = psmm.tile([128, 256], F32, tag="mm")
                    nc.tensor.matmul(
                        ps, wv_fr[0][:, t * 128 : (t + 1) * 128], owT[0],
                        start=True, stop=False,
                    )
                    nc.tensor.matmul(
                        ps, wv_fr[1][:, t * 128 : (t + 1) * 128], owT[1],
                        start=False, stop=True,
                    )
                    w2 = wpool.tile([128, 256], FR, tag=f"w2t{t}", name=f"w2t{t}")
                    nc.vector.tensor_copy(w2, ps)
                    w2t.append(w2)
                # ob_eff = out_b + out_w @ bv  (bv = qkv_b[512:768])
                ps_ob = psmm.tile([128, 2], F32, tag="mm")
                for m2 in range(2):
                    nc.tensor.matmul(
                        ps_ob[:, m2 : m2 + 1],
                        owT[0][:, m2 * 128 : (m2 + 1) * 128].bitcast(F32),
                        qb_sb[:, 4:5],
                        start=True, stop=False,
                    )
                    nc.tensor.matmul(
                        ps_ob[:, m2 : m2 + 1],
                        owT[1][:, m2 * 128 : (m2 + 1) * 128].bitcast(F32),
                        qb_sb[:, 5:6],
                        start=False, stop=True,
                    )
                ob_eff = stat.tile([128, 2], F32, tag="obeff")
                nc.vector.tensor_add(ob_eff, ps_ob, ob_sb)


                # ---------- GroupNorm stats ----------
                # per-channel raw sums: DVE does sum(x) while ACT does
                # sum(x^2) via Square+accum_out (parallel engines).
                st2 = []
                for t in range(2):
                    s1m = stat.tile([128, 4], F32, tag=f"s1m{t}", name=f"s1m{t}")
                    s2m = stat.tile([128, 4], F32, tag=f"s2m{t}", name=f"s2m{t}")
                    for cck in range(4):
                        sl = slice(cck * 1024, (cck + 1) * 1024)
                        nc.vector.reduce_sum(
                            out=s1m[:, cck : cck + 1],
                            in_=x_sb[t][:, sl].bitcast(F32),
                            axis=mybir.AxisListType.X,
                        )
                        sq = scpool.tile([128, 1024], F32, tag="sc", name=f"sq{t}{cck}")
                        nc.scalar.activation(
                            out=sq, in_=x_sb[t][:, sl].bitcast(F32),
                            func=AF.Square, accum_out=s2m[:, cck : cck + 1],
                        )
                    s2t = stat.tile([128, 2], F32, tag=f"st2{t}")
                    nc.vector.reduce_sum(out=s2t[:, 0:1], in_=s1m, axis=mybir.AxisListType.X)
                    nc.vector.reduce_sum(out=s2t[:, 1:2], in_=s2m, axis=mybir.AxisListType.X)
                    st2.append(s2t)
                psg = psmm.tile([8, 2], F32, tag="mm")
                nc.tensor.matmul(psg, gind[0], st2[0], start=True, stop=False)
                nc.tensor.matmul(psg, gind[1], st2[1], start=False, stop=True)
                gstat = stat.tile([8, 2], F32, tag="gstat")  # [mean_g, E[x^2]_g]
                nc.vector.tensor_scalar_mul(gstat, psg, 1.0 / (CPG * HW))
                var_g = stat.tile([8, 1], F32, tag="varg")
                nc.vector.tensor_mul(var_g, gstat[:, 0:1], gstat[:, 0:1])
                nc.vector.tensor_sub(var_g, gstat[:, 1:2], var_g)
                std_g = stat.tile([8, 1], F32, tag="stdg")
                nc.scalar.activation(out=std_g, in_=var_g, func=AF.Sqrt, bias=eps_sb, scale=1.0)
                rm = stat.tile([8, 2], F32, tag="rm")  # [rstd_g, mean_g]
                nc.vector.reciprocal(rm[:, 0:1], std_g)
                nc.vector.tensor_copy(rm[:, 1:2], gstat[:, 0:1])
                # broadcast to channels: [rstd_c, mean_c] = H_t.T @ rm
                ab = []
                for t in range(2):
                    psb = psmm.tile([128, 2], F32, tag="mm")
                    nc.tensor.matmul(psb, hind[t], rm, start=True, stop=True)
                    abt = stat.tile([128, 2], F32, tag=f"ab{t}")  # [a_c, b_c]
                    nc.vector.tensor_mul(abt[:, 0:1], psb[:, 0:1], g_sb[t])
                    nc.vector.tensor_mul(abt[:, 1:2], psb[:, 1:2], abt[:, 0:1])
                    nc.vector.tensor_sub(abt[:, 1:2], be_sb[t], abt[:, 1:2])
                    ab.append(abt)

                # ---------- fold GN into weights: no x_norm pass ----------
                # K/Q/V2 consume RAW x; W' = W * a (per c_in), biases get W@b.
                # Bias matmuls (plain fp32, N=1-2) use the UNSCALED weights;
                # the in-place scales below are WAR-ordered after them.
                ps_qb = psmm.tile([128, 4], F32, tag="mm", name="ps_qb")
                for m in range(4):
                    nc.tensor.matmul(
                        ps_qb[:, m : m + 1],
                        wT[0][:, m * 128 : (m + 1) * 128].bitcast(F32),
                        ab[0][:, 1:2],
                        start=True, stop=False,
                    )
                    nc.tensor.matmul(
                        ps_qb[:, m : m + 1],
                        wT[1][:, m * 128 : (m + 1) * 128].bitcast(F32),
                        ab[1][:, 1:2],
                        start=False, stop=True,
                    )
                qb_eff = stat.tile([128, 4], F32, tag="qbeff")
                nc.vector.tensor_add(qb_eff, ps_qb, qb_sb[:, 0:4])
                ps_ob2 = psmm.tile([128, 2], F32, tag="mm", name="ps_ob2")
                for m2 in range(2):
                    nc.tensor.matmul(
                        ps_ob2[:, m2 : m2 + 1],
                        w2t[0][:, m2 * 128 : (m2 + 1) * 128].bitcast(F32),
                        ab[0][:, 1:2],
                        start=True, stop=False,
                    )
                    nc.tensor.matmul(
                        ps_ob2[:, m2 : m2 + 1],
                        w2t[1][:, m2 * 128 : (m2 + 1) * 128].bitcast(F32),
                        ab[1][:, 1:2],
                        start=False, stop=True,
                    )
                ob_f = stat.tile([128, 2], F32, tag="obf")
                nc.vector.tensor_add(ob_f, ps_ob2, ob_eff)
                for t in range(2):
                    nc.vector.tensor_scalar_mul(wT[t], wT[t], ab[t][:, 0:1])
                    nc.vector.tensor_scalar_mul(w2t[t], w2t[t], ab[t][:, 0:1])

                # residual prep from raw xq bits
                y_sb = []
                for t in range(2):
                    yt = ypool.tile([128, NQ], F32, tag="y", name=f"y{t}")
                    nc.vector.tensor_scalar_add(
                        yt, xq_sb[t][:, :].bitcast(F32), ob_f[:, t : t + 1]
                    )
                    y_sb.append(yt)
                xn = x_sb
                xqn = xq_sb

                # ---------- qkv projections ----------
                q_sb = [qpool.tile([128, NQ], FR, tag="q", name=f"q{t}") for t in range(2)]
                k_sb = [kpool.tile([128, HW], FR, tag="k", name=f"k{t}") for t in range(2)]
                nch = 0
                for m in (2, 3, 0, 1):
                    dst = (q_sb, k_sb)[m // 2][m % 2]
                    src = xqn if m < 2 else xn
                    nj = NQ // 512 if m < 2 else HW // 512
                    for j in range(nj):
                        ps = psmm.tile([128, 512], F32, tag="mm")
                        nc.tensor.matmul(
                            ps,
                            wT[0][:, m * 128 : (m + 1) * 128],
                            src[0][:, j * 512 : (j + 1) * 512],
                            start=True,
                            stop=False,
                        )
                        nc.tensor.matmul(
                            ps,
                            wT[1][:, m * 128 : (m + 1) * 128],
                            src[1][:, j * 512 : (j + 1) * 512],
                            start=False,
                            stop=True,
                        )
                        dslice = dst[:, j * 512 : (j + 1) * 512]
                        if nch % 2 == 0:
                            nc.vector.tensor_scalar_add(dslice, ps, qb_eff[:, m : m + 1])
                        else:
                            nc.scalar.activation(
                                out=dslice, in_=ps, func=AF.Identity,
                                bias=qb_eff[:, m : m + 1], scale=1.0,
                            )
                        nch += 1

                # ---------- V2^T = xn^T @ W2^T (+ ones col) ----------
                NPAD = 16 if USE_FP8_PV else 2
                NV2 = 256 + NPAD
                v2dt = F8 if USE_FP8_PV else FR
                v2t = []
                for h in range(2):
                    v2 = vpool.tile([128, 16, NV2], v2dt, tag="v2", name=f"v2t{h}")
                    nc.gpsimd.dma_start(
                        out=v2[:, :, 256:NV2],
                        in_=ones_in[:, : 16 * NPAD].rearrange("p (f o) -> p f o", o=NPAD),
                    )
                    v2t.append(v2)
                for nt in range(32):
                    ps = psmm.tile([128, 256], F32, tag="mm")
                    nc.tensor.matmul(
                        ps, xn[0][:, nt * 128 : (nt + 1) * 128], w2t[0],
                        start=True, stop=False,
                    )
                    nc.tensor.matmul(
                        ps, xn[1][:, nt * 128 : (nt + 1) * 128], w2t[1],
                        start=False, stop=True,
                    )
                    dst = v2t[nt // 16][:, nt % 16, 0:256]
                    if nt % 2 == 0:
                        nc.vector.tensor_copy(dst, ps)
                    else:
                        nc.scalar.copy(dst, ps)

                if debug:
                    for t in range(2):
                        nc.sync.dma_start(
                            out=d_xn[t * 128 : (t + 1) * 128, :],
                            in_=xn[t][:, :].bitcast(F32),
                        )
                        nc.sync.dma_start(
                            out=d_q[t * 128 : (t + 1) * 128, :],
                            in_=q_sb[t][:, :].bitcast(F32),
                        )
                        nc.sync.dma_start(
                            out=d_k[t * 128 : (t + 1) * 128, :],
                            in_=k_sb[t][:, :].bitcast(F32),
                        )
                        nc.sync.dma_start(
                            out=d_ab[t * 128 : (t + 1) * 128, :], in_=ab[t]
                        )
                    for h in range(2):
                        nc.gpsimd.dma_start(
                            out=d_v2t.rearrange("(h f p) o -> h p f o", h=2, p=128)[h][:, :, :NV2],
                            in_=v2t[h][:, :, :] if USE_FP8_PV else v2t[h][:, :, :].bitcast(F32),
                        )

                # ---------- attention ----------
                for qc in range(NQ // 512):
                    po = [psov.tile([128, NV2], F32, tag="o", name=f"po{s_}") for s_ in range(4)]
                    if USE_FP8_PV:
                        for j in range(16):  # k-tile pairs
                            pT = ppool.tile([128, 2, 512], F8, tag="p", name="pT")
                            for i in range(2):
                                kt = 2 * j + i
                                ps = psmm.tile([128, 512], F32, tag="mm", name="ps")
                                nc.tensor.matmul(
                                    ps, k_sb[0][:, kt * 128 : (kt + 1) * 128],
                                    q_sb[0][:, qc * 512 : (qc + 1) * 512],
                                    start=True, stop=False,
                                )
                                nc.tensor.matmul(
                                    ps, k_sb[1][:, kt * 128 : (kt + 1) * 128],
                                    q_sb[1][:, qc * 512 : (qc + 1) * 512],
                                    start=False, stop=True,
                                )
                                nc.scalar.activation(
                                    out=pT[:, i, :], in_=ps, func=AF.Exp,
                                    scale=1.0 / 16.0, bias=ebias_sb,
                                )
                            rhs = v2t[j // 8][:, (j % 8) * 2 : (j % 8) * 2 + 2, :]
                            for s in range(4):
                                nc.tensor.matmul(
                                    po[s],
                                    pT[:, :, s * 128 : (s + 1) * 128],
                                    rhs,
                                    start=(j == 0),
                                    stop=(j == 15),
                                    perf_mode=mybir.MatmulPerfMode.DoubleRow,
                                    skip_group_check=True,
                                )
                    else:
                        for kt in range(32):
                            ps = psmm.tile([128, 512], F32, tag="mm")
                            nc.tensor.matmul(
                                ps, k_sb[0][:, kt * 128 : (kt + 1) * 128],
                                q_sb[0][:, qc * 512 : (qc + 1) * 512],
                                start=True, stop=False,
                            )
                            nc.tensor.matmul(
                                ps, k_sb[1][:, kt * 128 : (kt + 1) * 128],
                                q_sb[1][:, qc * 512 : (qc + 1) * 512],
                                start=False, stop=True,
                            )
                            pT = ppool.tile([128, 512], FR, tag="p")
                            nc.scalar.activation(
                                out=pT, in_=ps, func=AF.Exp, scale=1.0 / 16.0
                            )
                            for s in range(4):
                                nc.tensor.matmul(
                                    po[s],
                                    pT[:, s * 128 : (s + 1) * 128],
                                    v2t[kt // 16][:, kt % 16, :],
                                    start=(kt == 0),
                                    stop=(kt == 31),
                                    skip_group_check=True,
                                )
                    if debug and qc == 0:
                        dpo = opool.tile([128, NV2], F32, tag="dpo", name="dpo")
                        nc.vector.tensor_copy(dpo, po[0])
                        nc.sync.dma_start(out=d_po[:, :NV2], in_=dpo)
                    for s in range(4):
                        rl = small.tile([128, 1], F32, tag="rl")
                        nc.vector.reciprocal(rl, po[s][:, 256:257])
                        o_sb = opool.tile([128, 256], F32, tag="osb")
                        nc.scalar.activation(
                            out=o_sb, in_=po[s][:, 0:256],
                            func=AF.Copy, scale=rl,
                        )
                        for t in range(2):
                            pst = psmm.tile([128, 128], F32, tag="mm")
                            nc.tensor.transpose(
                                pst, o_sb[:, t * 128 : (t + 1) * 128], ident
                            )
                            ys = y_sb[t][:, qc * 512 + s * 128 : qc * 512 + (s + 1) * 128]
                            nc.vector.tensor_tensor(ys, pst, ys, ALU.add)
                    for t in range(2):
                        nc.sync.dma_start(
                            out=y[t * 128 : (t + 1) * 128, qc * 512 : (qc + 1) * 512],
                            in_=y_sb[t][:, qc * 512 : (qc + 1) * 512],
                        )

            consts = setup()
            for rep in range(loop_reps):
                body(rep, consts)

    return nc


def _get_runner(loop_reps=1):
    key = ("runner", loop_reps)
    if key not in _CACHE:
        nc = _build_nc(loop_reps)
        _CACHE[key] = nc
    return _CACHE[key]


K_USE_FP8 = USE_FP8_PV


def make_extra_inputs():
    gind = np.zeros((128, 16), dtype=np.float32)
    hind = np.zeros((8, 256), dtype=np.float32)
    for t in range(2):
        for p in range(128):
            g = (t * 128 + p) // CPG
            gind[p, t * 8 + g] = 1.0
            hind[g, t * 128 + p] = 1.0
    op = np.zeros((128, 256), dtype=np.float32)
    op[:, 0::16 if USE_FP8_PV else 2] = 1.0
    return {"gind_in": gind, "hind_in": hind, "ones_in": op,
            "ident_in": np.eye(128, dtype=np.float32)}


def kernel(x, gn_gamma, gn_beta, qkv_w, qkv_b, out_w, out_b):
    from concourse.bass_utils import run_bass_kernel_spmd

    x = np.asarray(x, dtype=np.float32)
    gn_gamma = np.asarray(gn_gamma, dtype=np.float32)
    gn_beta = np.asarray(gn_beta, dtype=np.float32)
    qkv_w = np.asarray(qkv_w, dtype=np.float32)
    qkv_b = np.asarray(qkv_b, dtype=np.float32)
    out_w = np.asarray(out_w, dtype=np.float32)
    out_b = np.asarray(out_b, dtype=np.float32)

    b, c, h, w = x.shape
    assert (b, c, h * w) == (B, C, HW)
    xf = x.reshape(b, c, HW)

    nc = _get_runner()
    in_maps = []
    for j in range(N_CORES):
        bi, qh = j // 2, j % 2
        if qh == 0:
            xbj = np.ascontiguousarray(xf[bi])
        else:
            xbj = np.concatenate([xf[bi][:, NQ:], xf[bi][:, :NQ]], axis=1)
        in_maps.append(
            {
                "xb": xbj,
                "qkv_w": qkv_w,
                "qkv_b": qkv_b,
                "out_w": out_w,
                "out_b": out_b,
                "gn_gamma": gn_gamma,
                "gn_beta": gn_beta,
            }
        )
    extras = make_extra_inputs()
    for m in in_maps:
        m.update(extras)
    res = run_bass_kernel_spmd(nc, in_maps, core_ids=list(range(N_CORES)))
    out = np.empty((B, C, HW), dtype=np.float32)
    for j in range(N_CORES):
        bi, qh = j // 2, j % 2
        out[bi][:, qh * NQ : (qh + 1) * NQ] = res.results[j]["y"]
    return out.reshape(b, c, h, w)
